# revision 8
# baseline (speedup 1.0000x reference)
"""Trainium2 Bass kernel for expected-calibration-error (ECE) over [N,C] logits.

Contract: kernel(logits, targets) -> np.float32 scalar (shape ()), matching

    probs = softmax(logits); conf = max(probs); pred = argmax(probs)
    acc = (pred == targets); bins of width 1/10 over (k/10, (k+1)/10]
    ECE = sum_k |avg_conf_k - avg_acc_k| * count_k / N

Strategy (data-parallel over 8 NeuronCores, rows sharded; v5):
  The per-row reductions are the whole cost. The max fold must live on
  the DVE (only engine with a 2x fp16 tree-fold); it runs in 64-column
  units to amortize instruction overhead. The SUMEXP is routed per
  column range across THREE mechanisms so no single resource binds
  (measured marginal costs per column: DVE fold+fast-exp ~129 ns,
  ACT fused exp+accum ~586 ns, xbar+PE ~160 ns of shared DMA-queue wall
  + ~70 ns PE):
    * PE route (96 cols): ONE batched XBAR transpose DMA per chunk
      (blockwise out[c,j*128+p]=in[p,j*128+c], HW-verified) makes E_T
      blocks; the tensor engine reduces each block with a stationary-
      weights matmul (lhsT = E_T block, rhs = ones[128,1]) writing S'
      into PSUM[p, j] -- aligned with the row layout, evacuated to SBUF
      by one tiny copy. XBAR traffic shares the 16 DMA queues with the
      input stream (~400GB/s per core total), which caps this route.
    * ACT route (64 cols): scalar-engine Exp with accum_out fuses exp
      and the row sum, one instruction per column.
    * DVE route (96 cols): Schraudolph fast-exp (tensor_scalar mult+add
      at the 4x perf mode writing int16 bits that reinterpret as fp16
      ~ K*exp(y); bit-exact-verified) + fp16 TT add-fold at 2x.
  All routes produce S' = K*exp-sum (K = E[(1+f)/2^f] = 1.0406845; ACT
  routes use bias=ln K) in one SBUF fp32 vector S_sb[p, j].
  conf' = exp(m) * recip(S') = conf / K; thresholds pre-scaled by 1/K on
  the host; finalize multiplies sum_conf back by K. Fast-exp noise
  +-0.26%: ECE rel err ~2e-4 on the real data (gate 2e-2).
  acc = (tl == rowmax), exact in fp16. Bin masks: DVE is_gt broadcast
  ({0,1}, 4 groups of 64 cols); histogram triples accumulate in PSUM via
  one tiny PE matmul per 128-row column; host differences the cumulative
  stats. Loads are issued per chunk so early chunks land early; chunk
  order starts with a DVE-route chunk to prime the pipeline and places
  the xbar chunks mid-stream.
"""

import numpy as np

# Problem constants (hardcoded per harness contract).
N = 262144
C = 128
P = 128          # SBUF partitions
NB = 10          # calibration bins
NCORES = 8
ROWS_PER_CORE = N // NCORES          # 32768
TW = ROWS_PER_CORE // P              # 256 row-blocks (columns) per core

# fast-exp constants: n = round(y*1024/ln2 + 15360) bit-viewed as fp16
FS = 1477.3197
FO = 15360.0
K_CORR = 1.0406845  # E[(1+f)/2^f], f ~ U(0,1); mean inflation of fast-exp

# chunk layout: (cols, sum_route, xbar_ring_or_None)
#   sum_route: 'pe' (xbar+PE), 'accum' (ACT fused), 'fold' (DVE fold)
CHUNKS = (
    (16, "fold", None),
    (16, "pe", "act"),
    (32, "pe", "sync"),
    (32, "pe", "sync"),
    (16, "pe", "act"),
    (32, "accum", None),
    (32, "accum", None),
    (32, "fold", None),
    (32, "fold", None),
    (16, "fold", None),
)
MAXF_UNIT = 64   # columns per max-fold chain
GW = 64          # phase-2 group width (columns)

_CACHE = {}


def build(chunks=CHUNKS, gw=GW):
    """Build the Bass module. Returns nc."""
    import concourse.bacc as bacc
    import concourse.tile as tile
    from concourse import mybir

    f32 = mybir.dt.float32
    f16 = mybir.dt.float16
    i16 = mybir.dt.int16
    Alu = mybir.AluOpType
    Act = mybir.ActivationFunctionType
    X = mybir.AxisListType.X

    widths = [c[0] for c in chunks]
    assert sum(widths) == TW
    assert TW % gw == 0
    ngroups = TW // gw
    lnK = float(np.log(K_CORR))
    offs = np.cumsum([0] + widths)

    nc = bacc.Bacc(trn_type="TRN2")

    y_d = nc.dram_tensor("y", [P, TW * C], f16, kind="ExternalInput")
    tl_d = nc.dram_tensor("tl", [P, TW], f16, kind="ExternalInput")
    thr_d = nc.dram_tensor("thr", [1, NB + 1], f16, kind="ExternalInput")
    out_d = nc.dram_tensor("gstats", [3, NB + 1], f32, kind="ExternalOutput")

    with tile.TileContext(nc) as tc:
        with (
            tc.tile_pool(name="yT", bufs=2) as yT_pool,
            tc.tile_pool(name="eT", bufs=3) as eT_pool,
            tc.tile_pool(name="fold", bufs=3) as f_pool,
            tc.tile_pool(name="grp", bufs=3) as grp_pool,
            tc.tile_pool(name="single", bufs=1) as single,
            tc.tile_pool(name="psum", bufs=1, space="PSUM") as psum_pool,
        ):
            y_rl = single.tile([P, TW * C], f16)   # full y resident (8.4MB)
            tl_all = single.tile([P, TW], f16)
            nc.sync.dma_start(out=tl_all[:], in_=tl_d[:])
            thr_sb = single.tile([P, NB + 1], f16)
            nc.sync.dma_start(out=thr_sb[:], in_=thr_d[:].partition_broadcast(P))

            m_all = single.tile([P, TW], f16)
            S_sb = single.tile([P, TW], f32)
            rhs3 = single.tile([P, 3, TW], f16)
            nc.gpsimd.memset(rhs3[:, 0, :], 1.0)
            ones_w = single.tile([P, 1], f16)
            nc.gpsimd.memset(ones_w[:], 1.0)
            lnK_ap = single.tile([P, 1], f32)
            nc.gpsimd.memset(lnK_ap[:], lnK)
            e_scr = single.tile([P, C], f16)   # ACT accum route scratch

            S_ps = psum_pool.tile([P, TW], f32)
            pstats = psum_pool.tile([3, NB + 1], f32)

            def max_fold(o, w):
                """DVE fp16 TT max chain at 2x over cols [o, o+w)."""
                y3 = y_rl[:, o * C : (o + w) * C].rearrange(
                    "p (t c) -> p t c", c=C
                )
                cur, hw_ = y3, C
                while hw_ > 8:
                    h = hw_ // 2
                    M = f_pool.tile([P, MAXF_UNIT * 64], f16, name="Mv")
                    M3 = M[:, : w * h].rearrange("p (t c) -> p t c", c=h)
                    nc.vector.tensor_tensor(
                        out=M3, in0=cur[:, :, 0:h], in1=cur[:, :, h:hw_],
                        op=Alu.max,
                    )
                    cur, hw_ = M3, h
                nc.vector.tensor_reduce(
                    out=m_all[:, o : o + w], in_=cur, axis=X, op=Alu.max
                )

            def sum_pe(o, w, xring):
                """xbar-transpose chunk; fast-exp on T layout; PE block sums."""
                yT = yT_pool.tile([P, 32 * 128], f16, name="yT")
                xeng = nc.sync if xring == "sync" else nc.scalar
                xeng.dma_start_transpose(
                    out=yT[:, : w * 128].rearrange("p (j q) -> p j q", q=128),
                    in_=y_rl[:, o * C : (o + w) * C],
                )
                eT = eT_pool.tile([P, 32 * 128], f16, name="eT")
                nc.vector.tensor_scalar(
                    out=eT[:, : w * 128].bitcast(i16),
                    in0=yT[:, : w * 128],
                    scalar1=FS, scalar2=FO, op0=Alu.mult, op1=Alu.add,
                )
                for j in range(w):
                    nc.tensor.matmul(
                        S_ps[:, o + j : o + j + 1],
                        eT[:, j * 128 : (j + 1) * 128],
                        ones_w[:],
                        start=True, stop=True,
                        skip_group_check=True,
                    )
                # evacuate to the unified SBUF S vector (tiny)
                nc.vector.tensor_copy(
                    out=S_sb[:, o : o + w], in_=S_ps[:, o : o + w]
                )

            def sum_accum(o, w):
                """ACT fused exp+rowsum, one activation per column."""
                for j in range(o, o + w):
                    nc.scalar.activation(
                        out=e_scr[:], in_=y_rl[:, j * C : (j + 1) * C],
                        func=Act.Exp, bias=lnK_ap[:],
                        accum_out=S_sb[:, j : j + 1],
                    )

            def sum_fold(o, w):
                """DVE fast-exp (row layout) + fp16 TT add-fold at 2x."""
                E = eT_pool.tile([P, 32 * 128], f16, name="Erl")
                nc.vector.tensor_scalar(
                    out=E[:, : w * C].bitcast(i16),
                    in0=y_rl[:, o * C : (o + w) * C],
                    scalar1=FS, scalar2=FO, op0=Alu.mult, op1=Alu.add,
                )
                E3 = E[:, : w * C].rearrange("p (t c) -> p t c", c=C)
                cur, hw_ = E3, C
                while hw_ > 8:
                    h = hw_ // 2
                    M = f_pool.tile([P, MAXF_UNIT * 64], f16, name="Mv")
                    M3 = M[:, : w * h].rearrange("p (t c) -> p t c", c=h)
                    nc.vector.tensor_tensor(
                        out=M3, in0=cur[:, :, 0:h], in1=cur[:, :, h:hw_],
                        op=Alu.add,
                    )
                    cur, hw_ = M3, h
                nc.vector.tensor_reduce(
                    out=S_sb[:, o : o + w], in_=cur, axis=X, op=Alu.add
                )

            def phase2(grp):
                c0, c1 = grp * gw, (grp + 1) * gw
                e_m = grp_pool.tile([P, gw], f32, name="em")
                nc.scalar.activation(out=e_m[:], in_=m_all[:, c0:c1],
                                     func=Act.Exp)
                rs = grp_pool.tile([P, gw], f32, name="rs")
                nc.vector.reciprocal_approx_fast(out=rs[:], in_=S_sb[:, c0:c1])
                nc.vector.tensor_tensor(
                    out=rhs3[:, 1, c0:c1], in0=e_m[:], in1=rs[:], op=Alu.mult
                )
                # acc: target logit attains the row max (exact in fp16)
                nc.vector.tensor_tensor(
                    out=rhs3[:, 2, c0:c1], in0=m_all[:, c0:c1],
                    in1=tl_all[:, c0:c1], op=Alu.is_equal,
                )
                # {0,1} cumulative bin masks via DVE is_gt broadcast
                g = grp_pool.tile([P, gw, NB + 1], f16, name="gv")
                cb = rhs3[:, 1, c0:c1].unsqueeze(2).broadcast_to(
                    [P, gw, NB + 1]
                )
                tb = thr_sb[:].unsqueeze(1).broadcast_to([P, gw, NB + 1])
                nc.vector.tensor_tensor(out=g[:], in0=cb, in1=tb, op=Alu.is_gt)
                # per-column cumulative histogram triples on PE
                for j in range(gw):
                    nc.tensor.matmul(
                        pstats[:],
                        rhs3[:, :, c0 + j],
                        g[:, j, :],
                        start=(grp == 0 and j == 0),
                        stop=(grp == ngroups - 1 and j == gw - 1),
                        skip_group_check=True,
                    )

            # emission: per-chunk load + route; max-folds in MAXF_UNIT ranges
            pending = 0
            done = 0
            maxf_done = 0
            for k, (w, route, xring) in enumerate(chunks):
                o = int(offs[k])
                nc.sync.dma_start(
                    out=y_rl[:, o * C : (o + w) * C],
                    in_=y_d[:, o * C : (o + w) * C],
                )
                if route == "pe":
                    sum_pe(o, w, xring)
                elif route == "accum":
                    sum_accum(o, w)
                else:
                    sum_fold(o, w)
                done += w
                while maxf_done + MAXF_UNIT <= done:
                    max_fold(maxf_done, MAXF_UNIT)
                    maxf_done += MAXF_UNIT
                while pending < ngroups and done >= (pending + 1) * gw + 16 \
                        and maxf_done >= (pending + 1) * gw + 16:
                    phase2(pending)
                    pending += 1
            while maxf_done < TW:
                max_fold(maxf_done, MAXF_UNIT)
                maxf_done += MAXF_UNIT
            while pending < ngroups:
                phase2(pending)
                pending += 1

            stats_sb = single.tile([3, NB + 1], f32)
            nc.vector.tensor_copy(out=stats_sb[:], in_=pstats[:])
            nc.sync.dma_start(out=out_d[:], in_=stats_sb[:])

    nc.compile()
    return nc


def prep_inputs(logits, targets, ncores=NCORES):
    """Convert + shard host inputs. Returns list of per-core in_maps."""
    l = np.asarray(logits, dtype=np.float32)
    tg = np.asarray(targets).astype(np.int64)
    n = l.shape[0]

    y16 = l.astype(np.float16)
    tl16 = y16[np.arange(n), tg]
    thr = (np.arange(NB + 1, dtype=np.float64) / NB / K_CORR).reshape(
        1, NB + 1
    ).astype(np.float16)

    rpc = n // ncores
    in_maps = []
    for k in range(ncores):
        yk = y16[k * rpc : (k + 1) * rpc].reshape(P, TW * C)
        tlk = tl16[k * rpc : (k + 1) * rpc].reshape(P, TW)
        in_maps.append(
            {"y": np.ascontiguousarray(yk), "tl": np.ascontiguousarray(tlk),
             "thr": thr}
        )
    return in_maps


def finalize(gstats_list, n=N):
    """Combine per-core cumulative [3, 11] {0,1}-mask stats into the ECE."""
    G = np.zeros((3, NB + 1), dtype=np.float64)
    for gs in gstats_list:
        G += gs.astype(np.float64)
    per = G[:, 0:NB] - G[:, 1 : NB + 1]
    counts, sum_conf, sum_acc = per[0], per[1] * K_CORR, per[2]
    safe = np.maximum(counts, 1.0)
    avg_conf = sum_conf / safe
    avg_acc = sum_acc / safe
    prop = counts / float(n)
    ece = np.where(counts > 0, np.abs(avg_conf - avg_acc) * prop, 0.0).sum()
    return np.array(ece, dtype=np.float32)


LAST_RESULTS = None  # BassKernelResults of the most recent kernel() call


def kernel(logits, targets):
    global LAST_RESULTS
    from concourse.bass_utils import run_bass_kernel_spmd

    key = (CHUNKS, GW)
    if key not in _CACHE:
        _CACHE[key] = build(CHUNKS, GW)
    nc = _CACHE[key]

    in_maps = prep_inputs(logits, targets)
    res = run_bass_kernel_spmd(nc, in_maps, core_ids=list(range(NCORES)))
    LAST_RESULTS = res
    return finalize([r["gstats"] for r in res.results])


# revision 12
# speedup vs baseline: 1.0067x; 1.0067x over previous
"""Trainium2 Bass kernel for expected-calibration-error (ECE) over [N,C] logits.

Contract: kernel(logits, targets) -> np.float32 scalar (shape ()), matching

    probs = softmax(logits); conf = max(probs); pred = argmax(probs)
    acc = (pred == targets); bins of width 1/10 over (k/10, (k+1)/10]
    ECE = sum_k |avg_conf_k - avg_acc_k| * count_k / N

Strategy (data-parallel over 8 NeuronCores, rows sharded; v5):
  The per-row reductions are the whole cost. The max fold must live on
  the DVE (only engine with a 2x fp16 tree-fold); it runs in 64-column
  units to amortize instruction overhead. The SUMEXP is routed per
  column range across THREE mechanisms so no single resource binds
  (measured marginal costs per column: DVE fold+fast-exp ~129 ns,
  ACT fused exp+accum ~586 ns, xbar+PE ~160 ns of shared DMA-queue wall
  + ~70 ns PE):
    * PE route (96 cols): ONE batched XBAR transpose DMA per chunk
      (blockwise out[c,j*128+p]=in[p,j*128+c], HW-verified) makes E_T
      blocks; the tensor engine reduces each block with a stationary-
      weights matmul (lhsT = E_T block, rhs = ones[128,1]) writing S'
      into PSUM[p, j] -- aligned with the row layout, evacuated to SBUF
      by one tiny copy. XBAR traffic shares the 16 DMA queues with the
      input stream (~400GB/s per core total), which caps this route.
    * ACT route (64 cols): scalar-engine Exp with accum_out fuses exp
      and the row sum, one instruction per column.
    * DVE route (96 cols): Schraudolph fast-exp (tensor_scalar mult+add
      at the 4x perf mode writing int16 bits that reinterpret as fp16
      ~ K*exp(y); bit-exact-verified) + fp16 TT add-fold at 2x.
  All routes produce S' = K*exp-sum (K = E[(1+f)/2^f] = 1.0406845; ACT
  routes use bias=ln K) in one SBUF fp32 vector S_sb[p, j].
  conf' = exp(m) * recip(S') = conf / K; thresholds pre-scaled by 1/K on
  the host; finalize multiplies sum_conf back by K. Fast-exp noise
  +-0.26%: ECE rel err ~2e-4 on the real data (gate 2e-2).
  acc = (tl == rowmax), exact in fp16. Bin masks: DVE is_gt broadcast
  ({0,1}, 4 groups of 64 cols); histogram triples accumulate in PSUM via
  one tiny PE matmul per 128-row column; host differences the cumulative
  stats. Loads are issued per chunk so early chunks land early; chunk
  order starts with a DVE-route chunk to prime the pipeline and places
  the xbar chunks mid-stream.
"""

import numpy as np

# Problem constants (hardcoded per harness contract).
N = 262144
C = 128
P = 128          # SBUF partitions
NB = 10          # calibration bins
NCORES = 8
ROWS_PER_CORE = N // NCORES          # 32768
TW = ROWS_PER_CORE // P              # 256 row-blocks (columns) per core

# fast-exp constants: n = round(y*1024/ln2 + 15360) bit-viewed as fp16
FS = 1477.3197
FO = 15360.0
K_CORR = 1.0406845  # E[(1+f)/2^f], f ~ U(0,1); mean inflation of fast-exp

# chunk layout: (cols, sum_route, exp_eng)
#   sum_route: 'pe' (xbar+PE; xbars always on the scalar ring so the
#   sync ring streams loads unimpeded), 'fold' (DVE add-fold)
#   exp_eng: 'dve' fast-exp TSP, 'act' big-instruction true Exp
CHUNKS = (
    (16, "fold", "dve"),
    (32, "pe", "dve"),
    (32, "pe", "act"),
    (32, "pe", "dve"),
    (32, "pe", "act"),
    (32, "fold", "act"),
    (32, "fold", "act"),
    (32, "fold", "dve"),
    (16, "fold", "dve"),
)
MAXF_UNIT = 64   # columns per max-fold chain
GW = 64          # phase-2 group width (columns)

_CACHE = {}


def build(chunks=CHUNKS, gw=GW):
    """Build the Bass module. Returns nc."""
    import concourse.bacc as bacc
    import concourse.tile as tile
    from concourse import mybir

    f32 = mybir.dt.float32
    f16 = mybir.dt.float16
    i16 = mybir.dt.int16
    Alu = mybir.AluOpType
    Act = mybir.ActivationFunctionType
    X = mybir.AxisListType.X

    widths = [c[0] for c in chunks]
    assert sum(widths) == TW
    assert TW % gw == 0
    ngroups = TW // gw
    lnK = float(np.log(K_CORR))
    offs = np.cumsum([0] + widths)

    nc = bacc.Bacc(trn_type="TRN2")

    y_d = nc.dram_tensor("y", [P, TW * C], f16, kind="ExternalInput")
    tl_d = nc.dram_tensor("tl", [P, TW], f16, kind="ExternalInput")
    thr_d = nc.dram_tensor("thr", [1, NB + 1], f16, kind="ExternalInput")
    out_d = nc.dram_tensor("gstats", [3, NB + 1], f32, kind="ExternalOutput")

    with tile.TileContext(nc) as tc:
        with (
            tc.tile_pool(name="yT", bufs=2) as yT_pool,
            tc.tile_pool(name="eT", bufs=3) as eT_pool,
            tc.tile_pool(name="fold", bufs=3) as f_pool,
            tc.tile_pool(name="grp", bufs=3) as grp_pool,
            tc.tile_pool(name="single", bufs=1) as single,
            tc.tile_pool(name="psum", bufs=1, space="PSUM") as psum_pool,
        ):
            y_rl = single.tile([P, TW * C], f16)   # full y resident (8.4MB)
            tl_all = single.tile([P, TW], f16)
            nc.sync.dma_start(out=tl_all[:], in_=tl_d[:])
            thr_sb = single.tile([P, NB + 1], f16)
            nc.sync.dma_start(out=thr_sb[:], in_=thr_d[:].partition_broadcast(P))

            m_all = single.tile([P, TW], f16)
            S_sb = single.tile([P, TW], f32)
            rhs3 = single.tile([P, 3, TW], f16)
            nc.gpsimd.memset(rhs3[:, 0, :], 1.0)
            ones_w = single.tile([P, 1], f16)
            nc.gpsimd.memset(ones_w[:], 1.0)
            lnK_ap = single.tile([P, 1], f32)
            nc.gpsimd.memset(lnK_ap[:], lnK)
            e_scr = single.tile([P, C], f16)   # ACT accum route scratch

            S_ps = psum_pool.tile([P, TW], f32)
            pstats = psum_pool.tile([3, NB + 1], f32)

            def max_fold(o, w):
                """DVE fp16 TT max chain at 2x over cols [o, o+w)."""
                y3 = y_rl[:, o * C : (o + w) * C].rearrange(
                    "p (t c) -> p t c", c=C
                )
                cur, hw_ = y3, C
                while hw_ > 8:
                    h = hw_ // 2
                    M = f_pool.tile([P, MAXF_UNIT * 64], f16, name="Mv")
                    M3 = M[:, : w * h].rearrange("p (t c) -> p t c", c=h)
                    nc.vector.tensor_tensor(
                        out=M3, in0=cur[:, :, 0:h], in1=cur[:, :, h:hw_],
                        op=Alu.max,
                    )
                    cur, hw_ = M3, h
                nc.vector.tensor_reduce(
                    out=m_all[:, o : o + w], in_=cur, axis=X, op=Alu.max
                )

            def exp_chunk(out_ap, in_ap, eng):
                """Elementwise K*exp: DVE fast-exp TSP or ACT true Exp."""
                if eng == "act":
                    nc.scalar.activation(
                        out=out_ap, in_=in_ap, func=Act.Exp, bias=lnK_ap[:],
                    )
                else:
                    nc.vector.tensor_scalar(
                        out=out_ap.bitcast(i16), in0=in_ap,
                        scalar1=FS, scalar2=FO, op0=Alu.mult, op1=Alu.add,
                    )

            def sum_pe(o, w, eng):
                """xbar-transpose chunk; exp on T layout; PE block sums."""
                yT = yT_pool.tile([P, 32 * 128], f16, name="yT")
                nc.scalar.dma_start_transpose(
                    out=yT[:, : w * 128].rearrange("p (j q) -> p j q", q=128),
                    in_=y_rl[:, o * C : (o + w) * C],
                )
                eT = eT_pool.tile([P, 32 * 128], f16, name="eT")
                exp_chunk(eT[:, : w * 128], yT[:, : w * 128], eng)
                for j in range(w):
                    nc.tensor.matmul(
                        S_ps[:, o + j : o + j + 1],
                        eT[:, j * 128 : (j + 1) * 128],
                        ones_w[:],
                        start=True, stop=True,
                        skip_group_check=True,
                    )
                # evacuate to the unified SBUF S vector (tiny)
                nc.vector.tensor_copy(
                    out=S_sb[:, o : o + w], in_=S_ps[:, o : o + w]
                )

            def sum_fold(o, w, eng):
                """exp (row layout) + DVE fp16 TT add-fold at 2x."""
                E = eT_pool.tile([P, 32 * 128], f16, name="Erl")
                exp_chunk(E[:, : w * C], y_rl[:, o * C : (o + w) * C], eng)
                E3 = E[:, : w * C].rearrange("p (t c) -> p t c", c=C)
                cur, hw_ = E3, C
                while hw_ > 8:
                    h = hw_ // 2
                    M = f_pool.tile([P, MAXF_UNIT * 64], f16, name="Mv")
                    M3 = M[:, : w * h].rearrange("p (t c) -> p t c", c=h)
                    nc.vector.tensor_tensor(
                        out=M3, in0=cur[:, :, 0:h], in1=cur[:, :, h:hw_],
                        op=Alu.add,
                    )
                    cur, hw_ = M3, h
                nc.vector.tensor_reduce(
                    out=S_sb[:, o : o + w], in_=cur, axis=X, op=Alu.add
                )

            def phase2(grp):
                c0, c1 = grp * gw, (grp + 1) * gw
                e_m = grp_pool.tile([P, gw], f32, name="em")
                nc.scalar.activation(out=e_m[:], in_=m_all[:, c0:c1],
                                     func=Act.Exp)
                rs = grp_pool.tile([P, gw], f32, name="rs")
                nc.vector.reciprocal_approx_fast(out=rs[:], in_=S_sb[:, c0:c1])
                nc.vector.tensor_tensor(
                    out=rhs3[:, 1, c0:c1], in0=e_m[:], in1=rs[:], op=Alu.mult
                )
                # acc: target logit attains the row max (exact in fp16)
                nc.vector.tensor_tensor(
                    out=rhs3[:, 2, c0:c1], in0=m_all[:, c0:c1],
                    in1=tl_all[:, c0:c1], op=Alu.is_equal,
                )
                # {0,1} cumulative bin masks via DVE is_gt broadcast
                g = grp_pool.tile([P, gw, NB + 1], f16, name="gv")
                cb = rhs3[:, 1, c0:c1].unsqueeze(2).broadcast_to(
                    [P, gw, NB + 1]
                )
                tb = thr_sb[:].unsqueeze(1).broadcast_to([P, gw, NB + 1])
                nc.vector.tensor_tensor(out=g[:], in0=cb, in1=tb, op=Alu.is_gt)
                # per-column cumulative histogram triples on PE
                for j in range(gw):
                    nc.tensor.matmul(
                        pstats[:],
                        rhs3[:, :, c0 + j],
                        g[:, j, :],
                        start=(grp == 0 and j == 0),
                        stop=(grp == ngroups - 1 and j == gw - 1),
                        skip_group_check=True,
                    )

            # emission: per-chunk load + route; max-folds in MAXF_UNIT ranges
            pending = 0
            done = 0
            maxf_done = 0
            for k, (w, route, eng) in enumerate(chunks):
                o = int(offs[k])
                nc.sync.dma_start(
                    out=y_rl[:, o * C : (o + w) * C],
                    in_=y_d[:, o * C : (o + w) * C],
                )
                if route == "pe":
                    sum_pe(o, w, eng)
                else:
                    sum_fold(o, w, eng)
                done += w
                while maxf_done + MAXF_UNIT <= done:
                    max_fold(maxf_done, MAXF_UNIT)
                    maxf_done += MAXF_UNIT
                while pending < ngroups and done >= (pending + 1) * gw + 16 \
                        and maxf_done >= (pending + 1) * gw + 16:
                    phase2(pending)
                    pending += 1
            while maxf_done < TW:
                max_fold(maxf_done, MAXF_UNIT)
                maxf_done += MAXF_UNIT
            while pending < ngroups:
                phase2(pending)
                pending += 1

            stats_sb = single.tile([3, NB + 1], f32)
            nc.vector.tensor_copy(out=stats_sb[:], in_=pstats[:])
            nc.sync.dma_start(out=out_d[:], in_=stats_sb[:])

    nc.compile()
    return nc


def prep_inputs(logits, targets, ncores=NCORES):
    """Convert + shard host inputs. Returns list of per-core in_maps."""
    l = np.asarray(logits, dtype=np.float32)
    tg = np.asarray(targets).astype(np.int64)
    n = l.shape[0]

    y16 = l.astype(np.float16)
    tl16 = y16[np.arange(n), tg]
    thr = (np.arange(NB + 1, dtype=np.float64) / NB / K_CORR).reshape(
        1, NB + 1
    ).astype(np.float16)

    rpc = n // ncores
    in_maps = []
    for k in range(ncores):
        yk = y16[k * rpc : (k + 1) * rpc].reshape(P, TW * C)
        tlk = tl16[k * rpc : (k + 1) * rpc].reshape(P, TW)
        in_maps.append(
            {"y": np.ascontiguousarray(yk), "tl": np.ascontiguousarray(tlk),
             "thr": thr}
        )
    return in_maps


def finalize(gstats_list, n=N):
    """Combine per-core cumulative [3, 11] {0,1}-mask stats into the ECE."""
    G = np.zeros((3, NB + 1), dtype=np.float64)
    for gs in gstats_list:
        G += gs.astype(np.float64)
    per = G[:, 0:NB] - G[:, 1 : NB + 1]
    counts, sum_conf, sum_acc = per[0], per[1] * K_CORR, per[2]
    safe = np.maximum(counts, 1.0)
    avg_conf = sum_conf / safe
    avg_acc = sum_acc / safe
    prop = counts / float(n)
    ece = np.where(counts > 0, np.abs(avg_conf - avg_acc) * prop, 0.0).sum()
    return np.array(ece, dtype=np.float32)


LAST_RESULTS = None  # BassKernelResults of the most recent kernel() call


def kernel(logits, targets):
    global LAST_RESULTS
    from concourse.bass_utils import run_bass_kernel_spmd

    key = (CHUNKS, GW)
    if key not in _CACHE:
        _CACHE[key] = build(CHUNKS, GW)
    nc = _CACHE[key]

    in_maps = prep_inputs(logits, targets)
    res = run_bass_kernel_spmd(nc, in_maps, core_ids=list(range(NCORES)))
    LAST_RESULTS = res
    return finalize([r["gstats"] for r in res.results])
